# revision 4
# baseline (speedup 1.0000x reference)
"""Bass/Trainium2 kernel for FLAOperator(mode='gla') CPU-fallback scan.

Reference recurrence (per b, h, d lane, over t = 0..N-1):
    s_t = s_{t-1} + sigmoid(q_t * k_t + g_t) * v_t ;  y_t = s_t
i.e. y = cumsum over N of u, with u = sigmoid(q*k + g) * v  (pure elementwise).

Shapes: q,k,v,g,y all [B=2, H=16, N=4096, D=128] f32.

Strategy (8 NeuronCores, SPMD, no collectives):
  - Shard the 32 independent (b,h) recurrences: 4 per core.
  - Per (b,h), process N in chunks of 2048 rows. Natural SBUF layout
    [128 part = n-within-block, free = (block, d)] so HBM DMAs move 1 MiB of
    contiguous data per tensor per chunk.
  - u computed elementwise in-place (DVE mult/add, ACT sigmoid, DVE mult).
  - 128x128 blocks of u are PE-transposed into PSUM as [d, n] so the DVE
    tensor_tensor_scan instruction (prefix scan along the free axis, fp32
    state, per-partition initial) produces the cumulative sum; chunks chain
    via initial = previous scan's last column.
  - Scanned [d, n] tiles are PE-transposed back to natural layout, copied
    PSUM->SBUF on ACT (DMA cannot read PSUM), and DMA'd out.
"""

from contextlib import ExitStack

import numpy as np

import concourse.bass as bass
import concourse.masks as masks
import concourse.tile as tile
from concourse import bacc, mybir
from concourse.bass_utils import run_bass_kernel_spmd

B, H, N, D = 2, 16, 4096, 128
N_CORES = 8
BH = B * H                    # 32 independent recurrences
BH_PER_CORE = BH // N_CORES   # 4
P = 128                       # partitions
CHUNK = 2048                  # n-rows per processing chunk (1 MiB DMAs)
NCHUNKS = N // CHUNK          # 2
SUB = 512                     # psum sub-chunk width (one PSUM bank of f32)
F32 = mybir.dt.float32

_PROGRAM = None       # cached compiled Bass program (module-level)
LAST_RESULTS = None   # BassKernelResults of the last run (for test harness)


def _build_program() -> bass.Bass:
    nc = bacc.Bacc("TRN2", debug=False, num_devices=N_CORES)

    q_d = nc.dram_tensor("q", [BH_PER_CORE, N, D], F32, kind="ExternalInput").ap()
    k_d = nc.dram_tensor("k", [BH_PER_CORE, N, D], F32, kind="ExternalInput").ap()
    v_d = nc.dram_tensor("v", [BH_PER_CORE, N, D], F32, kind="ExternalInput").ap()
    g_d = nc.dram_tensor("g", [BH_PER_CORE, N, D], F32, kind="ExternalInput").ap()
    y_d = nc.dram_tensor("y", [BH_PER_CORE, N, D], F32, kind="ExternalOutput").ap()

    with tile.TileContext(nc) as tc, ExitStack() as ctx:
        const_pool = ctx.enter_context(tc.tile_pool(name="const", bufs=1))
        io_pool = ctx.enter_context(tc.tile_pool(name="io", bufs=2))
        tmp_pool = ctx.enter_context(tc.tile_pool(name="tmp", bufs=2))
        yt_pool = ctx.enter_context(tc.tile_pool(name="yt", bufs=2))
        out_pool = ctx.enter_context(tc.tile_pool(name="out", bufs=2))
        psA_pool = ctx.enter_context(tc.tile_pool(name="psA", bufs=2, space="PSUM"))
        psB_pool = ctx.enter_context(tc.tile_pool(name="psB", bufs=2, space="PSUM"))

        identity = const_pool.tile([P, P], F32, tag="identity")
        masks.make_identity(nc, identity[:])
        ones = const_pool.tile([P, SUB], F32, tag="ones")
        nc.vector.memset(ones[:], 1.0)

        def dma_in(dst_tile, src_ap):
            # [CHUNK, D] DRAM region -> [128, CHUNK] SBUF tile laid out as
            # partition p = n % 128, free = (n // 128, d); every descriptor
            # moves a contiguous 512 B row.
            nc.sync.dma_start(
                out=dst_tile[:].rearrange("p (t d) -> p t d", d=D),
                in_=src_ap.rearrange("(t p) d -> p t d", p=P),
            )

        for bh in range(BH_PER_CORE):
            prev_tail = None  # AP of last scanned column of previous chunk
            for c in range(NCHUNKS):
                rows = slice(c * CHUNK, (c + 1) * CHUNK)
                qt = io_pool.tile([P, CHUNK], F32, tag="q")
                kt = io_pool.tile([P, CHUNK], F32, tag="k")
                vt = io_pool.tile([P, CHUNK], F32, tag="v")
                gt = io_pool.tile([P, CHUNK], F32, tag="g")
                dma_in(qt, q_d[bh, rows, :])
                dma_in(kt, k_d[bh, rows, :])
                dma_in(vt, v_d[bh, rows, :])
                dma_in(gt, g_d[bh, rows, :])

                # u = sigmoid(q*k + g) * v, computed in-place in `a`
                a = tmp_pool.tile([P, CHUNK], F32, tag="a")
                nc.vector.tensor_mul(a[:], qt[:], kt[:])
                nc.vector.tensor_add(a[:], a[:], gt[:])
                nc.scalar.activation(a[:], a[:], mybir.ActivationFunctionType.Sigmoid)
                nc.vector.tensor_mul(a[:], a[:], vt[:])

                yT = yt_pool.tile([P, CHUNK], F32, tag="yT")
                yout = out_pool.tile([P, CHUNK], F32, tag="yout")
                for s in range(CHUNK // SUB):
                    lo = s * SUB
                    # transpose 4 [128,128] blocks of u into [d, n] order
                    psA = psA_pool.tile([P, SUB], F32, tag="psA")
                    for t in range(SUB // P):
                        nc.tensor.transpose(
                            psA[:, t * P : (t + 1) * P],
                            a[:, lo + t * P : lo + (t + 1) * P],
                            identity[:],
                        )
                    # prefix-sum along n: state = state * 1 + u
                    initial = 0.0 if (c == 0 and s == 0) else prev_tail
                    nc.vector.tensor_tensor_scan(
                        yT[:, lo : lo + SUB],
                        ones[:],
                        psA[:],
                        initial,
                        op0=mybir.AluOpType.mult,
                        op1=mybir.AluOpType.add,
                    )
                    prev_tail = yT[:, lo + SUB - 1 : lo + SUB]
                    # transpose back to natural layout and stage for DMA
                    psB = psB_pool.tile([P, SUB], F32, tag="psB")
                    for t in range(SUB // P):
                        nc.tensor.transpose(
                            psB[:, t * P : (t + 1) * P],
                            yT[:, lo + t * P : lo + (t + 1) * P],
                            identity[:],
                        )
                    nc.scalar.copy(yout[:, lo : lo + SUB], psB[:])

                nc.sync.dma_start(
                    out=y_d[bh, rows, :].rearrange("(t p) d -> p t d", p=P),
                    in_=yout[:].rearrange("p (t d) -> p t d", d=D),
                )

    nc.compile()  # bacc backend: wait legalization, reg alloc, nop fusion
    return nc


def kernel(q: np.ndarray, k: np.ndarray, v: np.ndarray, g: np.ndarray) -> np.ndarray:
    global _PROGRAM, LAST_RESULTS
    if _PROGRAM is None:
        _PROGRAM = _build_program()

    def shard(x):
        x = np.ascontiguousarray(np.asarray(x, dtype=np.float32)).reshape(BH, N, D)
        return [np.ascontiguousarray(x[i * BH_PER_CORE : (i + 1) * BH_PER_CORE])
                for i in range(N_CORES)]

    qs, ks, vs, gs = shard(q), shard(k), shard(v), shard(g)
    in_maps = [
        {"q": qs[i], "k": ks[i], "v": vs[i], "g": gs[i]} for i in range(N_CORES)
    ]
    LAST_RESULTS = run_bass_kernel_spmd(_PROGRAM, in_maps, core_ids=list(range(N_CORES)))
    y = np.concatenate([r["y"] for r in LAST_RESULTS.results], axis=0)
    return y.reshape(B, H, N, D)
